# revision 1
# baseline (speedup 1.0000x reference)
"""Cost-volume kernel for Trainium2 (Bass/Tile), 8-core SPMD.

Problem: left/right features [B=2, C=32, H=128, W=256] f32.
Output [B, 2C=64, D=48, H, W] where for disparity d in [-8, 40):
  out[:, 0:C,  d+8, h, x] = left[:, :, h, x]   if 0 <= x-d < W else 0
  out[:, C:2C, d+8, h, x] = right[:, :, h, x-d] if 0 <= x-d < W else 0

Sharding: channels split 4-per-core (8 cores, identical program).
Each core builds the full disparity band for its 4 left + 4 right
channels. Pure data-movement kernel, bound by the HBM write rate of
the 96 MiB/core output.

Perf notes (from NTFF traces):
  - HWDGE (sync/scalar) DMA rings only engage 8 of the 16 SDMA
    engines; SWDGE (gpsimd) engages all 16. All big transfers go SWDGE.
  - Every store is a full-width DMA with contiguous 8 KiB/partition
    source rows (128 descriptors of 8 KiB), which sustains near line
    rate. Right-side shifted windows are materialized by DVE copies
    into contiguous staging buffers to keep descriptors at 8 KiB.
  - Zero padding is produced in SBUF (host-padded right image, SBUF
    memsets for left), never as thin strided DRAM writes.
  - The right input arrives host-padded so no SBUF memset gates the
    first staging copies; left-buffer prep is emitted lazily to keep
    the gpsimd DMA FIFO from head-of-line blocking at startup.
"""

import numpy as np

B, C, H, W = 2, 32, 128, 256
MIN_D, MAX_D = -8, 40
D = MAX_D - MIN_D  # 48
N_CORES = 8
CPC = C // N_CORES  # 4 channels of each image per core
BC = B * CPC  # 8 (b, c) pairs per core

PAD_L = 39  # covers max shift d=39 (offset = x - d + PAD_L >= 0)
PAD_R = 9   # covers min shift d=-8 (x - d <= 263 -> offset 302 < 304)
WP = PAD_L + W + PAD_R  # 304

HL = 8            # h rows held per partition
HH = H // HL      # 16
NPART = BC * HH   # 128 partitions: p = (b*CPC + c)*HH + h_hi

POS_BUFS = 4  # left work buffers for d >= 0 (buffer j: d = j, j+4, ... asc)
NEG_BUFS = 2  # left work buffers for d < 0 (buffer j: d = -(j+1), -(j+1)-2, ... desc)
STAGE_BUFS = 16  # right staging rotation depth (deep: keeps SDMA queues fed)

# Disabled (= MAX_D): writing only the valid columns [d, W) for large d
# and leaving the zero band to the runtime's pre-zeroed output buffers
# was measured SLOWER (348 us vs 298 us) — sub-1-KiB DRAM runs cost more
# in SWDGE descriptor/packet overhead than the ~2 MB of zeros they save.
DIRECT_D = MAX_D

# store order for the left side: negatives interleaved early; within a
# buffer positives ascend and negatives descend (zero regions only grow).
LEFT_ORDER = [0, -1, 1, -2, 2, 3, -3, 4, 5, -4, 6, 7, -5, 8, 9, -6, 10,
              11, -7, 12, 13, -8] + list(range(14, MAX_D))
assert sorted(LEFT_ORDER) == list(range(MIN_D, MAX_D))

_CACHE = {}


def _build_nc():
    import concourse.bacc as bacc
    import concourse.tile as tile
    import concourse.mybir as mybir

    f32 = mybir.dt.float32
    nc = bacc.Bacc(
        "TRN2",
        target_bir_lowering=False,
        debug=False,
        enable_asserts=False,
        num_devices=N_CORES,
    )
    left_in = nc.dram_tensor("left_in", [B, CPC, H, W], f32, kind="ExternalInput")
    right_in = nc.dram_tensor(
        "right_in", [B, CPC, H, WP], f32, kind="ExternalInput"
    )  # host-padded with zeros: data columns at [PAD_L, PAD_L + W)
    left_out = nc.dram_tensor(
        "left_out", [B, CPC, D, H, W], f32, kind="ExternalOutput"
    )
    right_out = nc.dram_tensor(
        "right_out", [B, CPC, D, H, W], f32, kind="ExternalOutput"
    )

    with tile.TileContext(nc) as tc:
        with (
            tc.tile_pool(name="pool", bufs=1) as pool,
            tc.tile_pool(name="stpool", bufs=STAGE_BUFS) as stpool,
        ):
            # ---- right image (pre-padded), loaded once ----
            rp = pool.tile([NPART, HL * WP], f32, tag="rp")
            rp3 = rp[:].rearrange("p (h w) -> p h w", h=HL)
            # zero source for left-edge zeroing, done as ACT copies so the
            # WAR-gated zeroing never head-of-line blocks the in-order DVE
            # queue that feeds the right-side staging copies
            zt = pool.tile([NPART, HL * max(POS_BUFS, NEG_BUFS)], f32, tag="zt")
            zt3 = zt[:].rearrange("p (h w) -> p h w", h=HL)
            nc.vector.memset(zt[:], 0.0)

            def zero_cols(t3, a, b):
                nc.scalar.copy(t3[:, :, a:b], zt3[:, :, 0 : b - a])

            # ---- left work buffers; pos[0] is the load target, the rest
            # are DVE-copied lazily on first use ----
            pos = []
            neg = []
            for j in range(POS_BUFS):
                t = pool.tile([NPART, HL * W], f32, tag=f"lp{j}")
                pos.append([t, t[:].rearrange("p (h w) -> p h w", h=HL), False])
            for j in range(NEG_BUFS):
                t = pool.tile([NPART, HL * W], f32, tag=f"ln{j}")
                neg.append([t, t[:].rearrange("p (h w) -> p h w", h=HL), False])
            pos[0][2] = True  # loaded directly, no copy needed
            # left load first: the d=0 left store depends only on it, so it
            # is the earliest possible store; right stores additionally
            # need a DVE staging copy after the right load lands
            nc.gpsimd.dma_start(pos[0][0][:], left_in.ap())
            nc.gpsimd.dma_start(rp[:], right_in.ap())

            def emit_left(d):
                if d >= DIRECT_D:
                    # valid columns only; zero band stays pre-zeroed DRAM.
                    # pos[0]'s work-buffer cycle only zeroes columns
                    # [0, DIRECT_D - POS_BUFS), disjoint from [d, W).
                    nc.gpsimd.dma_start(
                        left_out.ap()[:, :, d - MIN_D, :, d:W],
                        pos[0][1][:, :, d:W],
                    )
                    return
                if d >= 0:
                    buf = pos[d % POS_BUFS]
                    assert d < DIRECT_D
                    t, t3, ready = buf
                    if not ready:
                        nc.scalar.copy(t[:], pos[0][0][:])
                        if d > 0:
                            zero_cols(t3, 0, d)
                        buf[2] = True
                    elif d >= POS_BUFS:
                        zero_cols(t3, d - POS_BUFS, d)
                else:
                    buf = neg[(-d - 1) % NEG_BUFS]
                    t, t3, ready = buf
                    if not ready:
                        nc.scalar.copy(t[:], pos[0][0][:])
                        zero_cols(t3, W + d, W)
                        buf[2] = True
                    else:
                        zero_cols(t3, W + d, W + d + NEG_BUFS)
                nc.gpsimd.dma_start(left_out.ap()[:, :, d - MIN_D, :, :], t[:])

            def emit_right(di):
                d = di + MIN_D
                a = PAD_L - d
                stage = stpool.tile([NPART, HL * W], f32, tag="st")
                st3 = stage[:].rearrange("p (h w) -> p h w", h=HL)
                nc.vector.tensor_copy(st3[:], rp3[:, :, a : a + W])
                nc.gpsimd.dma_start(right_out.ap()[:, :, di, :, :], stage[:])

            emit_left(LEFT_ORDER[0])
            emit_left(LEFT_ORDER[1])
            for step in range(D):
                emit_right(step)
                if step + 2 < D:
                    emit_left(LEFT_ORDER[step + 2])

    nc.compile()
    return nc


def _get_nc():
    if "nc" not in _CACHE:
        _CACHE["nc"] = _build_nc()
    return _CACHE["nc"]


def kernel(left_feat, right_feat):
    from concourse.bass_utils import run_bass_kernel_spmd

    left = np.ascontiguousarray(np.asarray(left_feat), dtype=np.float32)
    right = np.ascontiguousarray(np.asarray(right_feat), dtype=np.float32)
    assert left.shape == (B, C, H, W) and right.shape == (B, C, H, W)

    nc = _get_nc()
    right_pad = np.zeros((B, C, H, WP), dtype=np.float32)
    right_pad[:, :, :, PAD_L : PAD_L + W] = right
    in_maps = []
    for m in range(N_CORES):
        sl = slice(m * CPC, (m + 1) * CPC)
        in_maps.append(
            {
                "left_in": np.ascontiguousarray(left[:, sl]),
                "right_in": np.ascontiguousarray(right_pad[:, sl]),
            }
        )
    res = run_bass_kernel_spmd(nc, in_maps, core_ids=list(range(N_CORES))).results

    out = np.empty((B, 2 * C, D, H, W), dtype=np.float32)
    for m in range(N_CORES):
        sl = slice(m * CPC, (m + 1) * CPC)
        out[:, sl] = res[m]["left_out"]
        out[:, C + m * CPC : C + (m + 1) * CPC] = res[m]["right_out"]
    return out



# revision 7
# speedup vs baseline: 1.2156x; 1.2156x over previous
"""Cost-volume kernel for Trainium2 (Bass/Tile), 8-core SPMD.

Problem: left/right features [B=2, C=32, H=128, W=256] f32.
Output [B, 2C=64, D=48, H, W] where for disparity d in [-8, 40):
  out[:, 0:C,  d+8, h, x] = left[:, :, h, x]   if 0 <= x-d < W else 0
  out[:, C:2C, d+8, h, x] = right[:, :, h, x-d] if 0 <= x-d < W else 0

Sharding: channels split 4-per-core (8 cores, identical program).
Each core builds the full disparity band for its 4 left + 4 right
channels. Pure data-movement kernel, bound by the HBM write rate of
the 96 MiB/core output (per-NC HBM cap ~358 GB/s; steady state runs
within ~6% of it).

v2 changes vs the 298us baseline (all from NTFF trace analysis):
  - The first ~10us of the program are dead (framework preamble + Q7
    SWDGE spin-up before the first descriptor lands). The two
    shift-free output slices (left d=0, right d=0 -> di=8) are issued
    as DRAM->DRAM HWDGE copies on the sync queue, which needs no SBUF
    staging and no Q7, so they fill the startup hole with real output
    bytes.
  - Stores are batched: one SWDGE dma_start covers 2 (left) or up to
    4 (right) disparity slices via a permuted dest AP (d-slot dim
    placed between the partition row (b,c,h_hi) and the contiguous
    8 KiB (h_lo,w) run). Descriptor shape is unchanged -- still 8 KiB
    per-partition runs -- but instruction count drops ~96 -> ~40,
    which removes most of the per-instruction 4-byte semaphore
    packets (~13.8k of them = ~8us/engine in the baseline trace) and
    Q7 dispatch work.
  - Both images are kept host-padded in SBUF; every left work-slot
    init copies from the pristine padded left image (the baseline
    sourced them from a work buffer that was already being zeroed,
    which forced a rigid emission order).
"""

import numpy as np

B, C, H, W = 2, 32, 128, 256
MIN_D, MAX_D = -8, 40
D = MAX_D - MIN_D  # 48
N_CORES = 8
CPC = C // N_CORES  # 4 channels of each image per core
BC = B * CPC  # 8 (b, c) pairs per core

PAD_L = 39  # covers max shift d=39 (offset = x - d + PAD_L >= 0)
PAD_R = 9   # covers min shift d=-8 (x - d <= 263 -> offset 302 < 304)
WP = PAD_L + W + PAD_R  # 304

HL = 8            # h rows held per partition
HH = H // HL      # 16
NPART = BC * HH   # 128 partitions: p = (b*CPC + c)*HH + h_hi

STAGE_BUFS = 3  # right staging rotation depth (each buf holds up to 4 d)

_CACHE = {}


def _build_nc():
    import concourse.bacc as bacc
    import concourse.tile as tile
    import concourse.mybir as mybir

    f32 = mybir.dt.float32
    nc = bacc.Bacc(
        "TRN2",
        target_bir_lowering=False,
        debug=False,
        enable_asserts=False,
        num_devices=N_CORES,
    )
    # host-padded images: data columns at [PAD_L, PAD_L + W), zeros outside
    left_in = nc.dram_tensor("left_in", [B, CPC, H, WP], f32, kind="ExternalInput")
    right_in = nc.dram_tensor("right_in", [B, CPC, H, WP], f32, kind="ExternalInput")
    # raw (unpadded) copies, only for the DRAM->DRAM prologue stores
    left_raw = nc.dram_tensor("left_raw", [B, CPC, H, W], f32, kind="ExternalInput")
    right_raw = nc.dram_tensor("right_raw", [B, CPC, H, W], f32, kind="ExternalInput")
    # disparity-major outputs: each d-slice is one contiguous 1 MiB run,
    # so a batched multi-d store AP merges to 3 dims (partition row, d,
    # 8 KiB run) -- the DMA AP balancer rejects >3 dims. Host transposes
    # back when gathering.
    left_out = nc.dram_tensor(
        "left_out", [D, B, CPC, H, W], f32, kind="ExternalOutput"
    )
    right_out = nc.dram_tensor(
        "right_out", [D, B, CPC, H, W], f32, kind="ExternalOutput"
    )

    def dest(out_t, di0, g):
        # dest AP for g consecutive disparity slices starting at di0,
        # iterated (b, c, h_hi, d, h_lo, w) to match the SBUF source
        # (partition = (b,c,h_hi), columns = (d_slot, h_lo, w)).
        ap = out_t.ap()[di0 : di0 + g, :, :, :, :]
        return ap.rearrange("g b c (hh hl) w -> b c hh g hl w", hl=HL)

    with tile.TileContext(nc) as tc:
        with (
            tc.tile_pool(name="pool", bufs=1) as pool,
            tc.tile_pool(name="stpool", bufs=STAGE_BUFS) as stpool,
        ):
            # ---- DRAM->DRAM prologue: the two shift-free slices ----
            # No SBUF dependency; runs on the HWDGE (sync queue) during
            # the otherwise-dead SWDGE spin-up + input-load window.
            nc.sync.dma_start(left_out.ap()[0 - MIN_D], left_raw.ap())
            nc.sync.dma_start(right_out.ap()[0 - MIN_D], right_raw.ap())

            # ---- tiles ----
            lpimg = pool.tile([NPART, HL * WP], f32, tag="lpimg")
            lpimg3 = lpimg[:].rearrange("p (h w) -> p h w", h=HL)
            rp = pool.tile([NPART, HL * WP], f32, tag="rp")
            rp3 = rp[:].rearrange("p (h w) -> p h w", h=HL)
            zt = pool.tile([NPART, HL * 8], f32, tag="zt")
            zt3 = zt[:].rearrange("p (h w) -> p h w", h=HL)

            # left work tiles: 2 pos (A, B), 2 neg (C, D), 2 d-slots each
            lp = [
                pool.tile([NPART, 2 * HL * W], f32, tag=f"lp{j}", name=f"lp{j}")
                for j in range(2)
            ]
            ln = [
                pool.tile([NPART, 2 * HL * W], f32, tag=f"ln{j}", name=f"ln{j}")
                for j in range(2)
            ]
            lp3 = [t[:].rearrange("p (g h w) -> p g h w", g=2, h=HL) for t in lp]
            ln3 = [t[:].rearrange("p (g h w) -> p g h w", g=2, h=HL) for t in ln]

            # ---- loads (SWDGE, all 16 engines) ----
            nc.gpsimd.dma_start(lpimg[:], left_in.ap())
            nc.gpsimd.dma_start(rp[:], right_in.ap())
            nc.vector.memset(zt[:], 0.0)

            def zero_cols(t4, g, a, b):
                nc.scalar.copy(t4[:, g, :, a:b], zt3[:, :, 0 : b - a])

            ready = set()

            def prep_slot(t4, key, g, za, zb):
                # ensure slot holds the left image with cols [za2, zb)
                # zeroed; on first use copy pristine image then zero
                # [0/za.., zb) as passed by the caller.
                if key not in ready:
                    nc.scalar.copy(t4[:, g, :, :], lpimg3[:, :, PAD_L : PAD_L + W])
                    ready.add(key)
                zero_cols(t4, g, za, zb)

            def emit_left_pair(i):
                # covers d = (2i+1, 2i+2); tile A (i even) / B (i odd)
                d0 = 2 * i + 1
                j = i % 2
                for g, d in ((0, d0), (1, d0 + 1)):
                    za = 0 if ("p", j, g) not in ready else d - 4
                    prep_slot(lp3[j], ("p", j, g), g, za, d)
                nc.gpsimd.dma_start(dest(left_out, d0 - MIN_D, 2), lp[j][:])

            def emit_left_single39():
                d = 39  # tile B slot0: previous use d=35, zero [35,39)
                prep_slot(lp3[1], ("p", 1, 0), 0, d - 4, d)
                nc.gpsimd.dma_start(
                    left_out.ap()[d - MIN_D], lp[1][:, 0 : HL * W]
                )

            def emit_neg_pair(i):
                # covers d = (-2i-2, -2i-1) ascending; tile C/D; slot0
                # holds the more-negative d so dest d stays ascending.
                d0 = -2 * i - 2
                j = i % 2
                for g, d in ((0, d0), (1, d0 + 1)):
                    if ("n", j, g) not in ready:
                        prep_slot(ln3[j], ("n", j, g), g, W + d, W)
                    else:
                        zero_cols(ln3[j], g, W + d, W + d + 4)
                nc.gpsimd.dma_start(dest(left_out, d0 - MIN_D, 2), ln[j][:])

            def emit_right_batch(di0, g):
                st = stpool.tile([NPART, 4 * HL * W], f32, tag="st")
                st4 = st[:].rearrange("p (g h w) -> p g h w", g=4, h=HL)
                for k in range(g):
                    a = PAD_L - (di0 + k + MIN_D)
                    nc.vector.tensor_copy(st4[:, k, :, :], rp3[:, :, a : a + W])
                nc.gpsimd.dma_start(
                    dest(right_out, di0, g),
                    st[:, 0 : g * HL * W],
                )

            # ---- emission schedule ----
            # Early runway: left pairs depend only on the left load +
            # ACT copies, so they keep the engines busy while the
            # right image lands and the first staging copies run.
            # di=8 (d=0) is covered by the DRAM->DRAM prologue.
            emit_left_pair(0)       # d 1,2
            emit_left_pair(1)       # d 3,4
            rights = [(0, 2), (2, 2), (4, 2), (6, 2), (9, 4), (13, 4), (17, 4),
                      (21, 4), (25, 4), (29, 4), (33, 4), (37, 4), (41, 4), (45, 3)]
            lefts = (
                [("N", 0), ("P", 2), ("N", 1)]
                + [("P", i) for i in range(3, 19)]
                + [("S", None), ("N", 2), ("N", 3)]
            )
            li, ri = 0, 0
            step = 0
            while li < len(lefts) or ri < len(rights):
                if ri < len(rights):
                    emit_right_batch(*rights[ri])
                    ri += 1
                for _ in range(2 if step % 2 == 1 else 1):
                    if li < len(lefts):
                        kind, i = lefts[li]
                        if kind == "P":
                            emit_left_pair(i)
                        elif kind == "S":
                            emit_left_single39()
                        else:
                            emit_neg_pair(i)
                        li += 1
                step += 1

    nc.compile()
    return nc


def _get_nc():
    if "nc" not in _CACHE:
        _CACHE["nc"] = _build_nc()
    return _CACHE["nc"]


def kernel(left_feat, right_feat):
    from concourse.bass_utils import run_bass_kernel_spmd

    left = np.ascontiguousarray(np.asarray(left_feat), dtype=np.float32)
    right = np.ascontiguousarray(np.asarray(right_feat), dtype=np.float32)
    assert left.shape == (B, C, H, W) and right.shape == (B, C, H, W)

    nc = _get_nc()
    left_pad = np.zeros((B, C, H, WP), dtype=np.float32)
    left_pad[:, :, :, PAD_L : PAD_L + W] = left
    right_pad = np.zeros((B, C, H, WP), dtype=np.float32)
    right_pad[:, :, :, PAD_L : PAD_L + W] = right
    in_maps = []
    for m in range(N_CORES):
        sl = slice(m * CPC, (m + 1) * CPC)
        in_maps.append(
            {
                "left_in": np.ascontiguousarray(left_pad[:, sl]),
                "right_in": np.ascontiguousarray(right_pad[:, sl]),
                "left_raw": np.ascontiguousarray(left[:, sl]),
                "right_raw": np.ascontiguousarray(right[:, sl]),
            }
        )
    res = run_bass_kernel_spmd(nc, in_maps, core_ids=list(range(N_CORES))).results

    out = np.empty((B, 2 * C, D, H, W), dtype=np.float32)
    for m in range(N_CORES):
        sl = slice(m * CPC, (m + 1) * CPC)
        out[:, sl] = res[m]["left_out"].transpose(1, 2, 0, 3, 4)
        out[:, C + m * CPC : C + (m + 1) * CPC] = res[m]["right_out"].transpose(
            1, 2, 0, 3, 4
        )
    return out


# revision 11
# speedup vs baseline: 1.2199x; 1.0035x over previous
"""Cost-volume kernel for Trainium2 (Bass/Tile), 8-core SPMD.

Problem: left/right features [B=2, C=32, H=128, W=256] f32.
Output [B, 2C=64, D=48, H, W] where for disparity d in [-8, 40):
  out[:, 0:C,  d+8, h, x] = left[:, :, h, x]   if 0 <= x-d < W else 0
  out[:, C:2C, d+8, h, x] = right[:, :, h, x-d] if 0 <= x-d < W else 0

Sharding: channels split 4-per-core (8 cores, identical program).
Each core builds the full disparity band for its 4 left + 4 right
channels. Pure data-movement kernel, bound by the HBM write rate of
the 96 MiB/core output (per-NC HBM cap ~358 GB/s; steady state runs
within ~6% of it).

v2 changes vs the 298us baseline (all from NTFF trace analysis):
  - The first ~10us of the program are dead (framework preamble + Q7
    SWDGE spin-up before the first descriptor lands). The two
    shift-free output slices (left d=0, right d=0 -> di=8) are issued
    as DRAM->DRAM HWDGE copies on the sync queue, which needs no SBUF
    staging and no Q7, so they fill the startup hole with real output
    bytes.
  - Stores are batched: one SWDGE dma_start covers 2 (left) or up to
    4 (right) disparity slices via a permuted dest AP (d-slot dim
    placed between the partition row (b,c,h_hi) and the contiguous
    8 KiB (h_lo,w) run). Descriptor shape is unchanged -- still 8 KiB
    per-partition runs -- but instruction count drops ~96 -> ~40,
    which removes most of the per-instruction 4-byte semaphore
    packets (~13.8k of them = ~8us/engine in the baseline trace) and
    Q7 dispatch work.
  - Both images are kept host-padded in SBUF; every left work-slot
    init copies from the pristine padded left image (the baseline
    sourced them from a work buffer that was already being zeroed,
    which forced a rigid emission order).
"""

import numpy as np

B, C, H, W = 2, 32, 128, 256
MIN_D, MAX_D = -8, 40
D = MAX_D - MIN_D  # 48
N_CORES = 8
CPC = C // N_CORES  # 4 channels of each image per core
BC = B * CPC  # 8 (b, c) pairs per core

PAD_L = 39  # covers max shift d=39 (offset = x - d + PAD_L >= 0)
PAD_R = 9   # covers min shift d=-8 (x - d <= 263 -> offset 302 < 304)
WP = PAD_L + W + PAD_R  # 304

HL = 8            # h rows held per partition
HH = H // HL      # 16
NPART = BC * HH   # 128 partitions: p = (b*CPC + c)*HH + h_hi

STAGE_BUFS = 3  # right staging rotation depth (each buf holds up to 4 d)

_CACHE = {}


def _build_nc():
    import concourse.bacc as bacc
    import concourse.tile as tile
    import concourse.mybir as mybir

    f32 = mybir.dt.float32
    nc = bacc.Bacc(
        "TRN2",
        target_bir_lowering=False,
        debug=False,
        enable_asserts=False,
        num_devices=N_CORES,
    )
    # host-padded images: data columns at [PAD_L, PAD_L + W), zeros outside
    left_in = nc.dram_tensor("left_in", [B, CPC, H, WP], f32, kind="ExternalInput")
    right_in = nc.dram_tensor("right_in", [B, CPC, H, WP], f32, kind="ExternalInput")
    # raw (unpadded) copies, only for the DRAM->DRAM prologue stores
    left_raw = nc.dram_tensor("left_raw", [B, CPC, H, W], f32, kind="ExternalInput")
    right_raw = nc.dram_tensor("right_raw", [B, CPC, H, W], f32, kind="ExternalInput")
    # disparity-major outputs: each d-slice is one contiguous 1 MiB run,
    # so a batched multi-d store AP merges to 3 dims (partition row, d,
    # 8 KiB run) -- the DMA AP balancer rejects >3 dims. Host transposes
    # back when gathering.
    left_out = nc.dram_tensor(
        "left_out", [D, B, CPC, H, W], f32, kind="ExternalOutput"
    )
    right_out = nc.dram_tensor(
        "right_out", [D, B, CPC, H, W], f32, kind="ExternalOutput"
    )

    def dest(out_t, di0, g):
        # dest AP for g consecutive disparity slices starting at di0,
        # iterated (b, c, h_hi, d, h_lo, w) to match the SBUF source
        # (partition = (b,c,h_hi), columns = (d_slot, h_lo, w)).
        ap = out_t.ap()[di0 : di0 + g, :, :, :, :]
        return ap.rearrange("g b c (hh hl) w -> b c hh g hl w", hl=HL)

    with tile.TileContext(nc) as tc:
        with (
            tc.tile_pool(name="pool", bufs=1) as pool,
            tc.tile_pool(name="stpool", bufs=STAGE_BUFS) as stpool,
        ):
            # ---- DRAM->DRAM prologue: the two shift-free slices ----
            # No SBUF dependency; runs on the HWDGE (sync queue) during
            # the otherwise-dead SWDGE spin-up + input-load window.
            # 8 KiB descriptors: a 64 KiB descriptor occupies an SDMA
            # engine for ~3us per packet, starving the input loads in
            # the queue round-robin (engines switch queues only at
            # packet boundaries).
            nc.sync.dma_start(
                left_out.ap()[0 - MIN_D], left_raw.ap(), max_dma_last_dim=2048
            )
            nc.sync.dma_start(
                right_out.ap()[0 - MIN_D], right_raw.ap(), max_dma_last_dim=2048
            )

            # ---- tiles ----
            lpimg = pool.tile([NPART, HL * WP], f32, tag="lpimg")
            lpimg3 = lpimg[:].rearrange("p (h w) -> p h w", h=HL)
            rp = pool.tile([NPART, HL * WP], f32, tag="rp")
            rp3 = rp[:].rearrange("p (h w) -> p h w", h=HL)
            zt = pool.tile([NPART, HL * 8], f32, tag="zt")
            zt3 = zt[:].rearrange("p (h w) -> p h w", h=HL)

            # left work tiles: 2 pos (A, B), 2 neg (C, D), 2 d-slots each
            lp = [
                pool.tile([NPART, 2 * HL * W], f32, tag=f"lp{j}", name=f"lp{j}")
                for j in range(2)
            ]
            ln = [
                pool.tile([NPART, 2 * HL * W], f32, tag=f"ln{j}", name=f"ln{j}")
                for j in range(2)
            ]
            lp3 = [t[:].rearrange("p (g h w) -> p g h w", g=2, h=HL) for t in lp]
            ln3 = [t[:].rearrange("p (g h w) -> p g h w", g=2, h=HL) for t in ln]

            # ---- loads (SWDGE, all 16 engines) ----
            nc.gpsimd.dma_start(lpimg[:], left_in.ap())
            nc.gpsimd.dma_start(rp[:], right_in.ap())
            nc.vector.memset(zt[:], 0.0)

            def zero_cols(t4, g, a, b):
                nc.scalar.copy(t4[:, g, :, a:b], zt3[:, :, 0 : b - a])

            ready = set()

            def prep_slot(t4, key, g, za, zb, eng="scalar"):
                # ensure slot holds the left image with cols [za2, zb)
                # zeroed; on first use copy pristine image then zero
                # [0/za.., zb) as passed by the caller. The pos-tile
                # init copies are on DVE (faster, and ACT is serialized
                # behind them on the startup critical path otherwise).
                if key not in ready:
                    cp = nc.vector.tensor_copy if eng == "vector" else nc.scalar.copy
                    cp(t4[:, g, :, :], lpimg3[:, :, PAD_L : PAD_L + W])
                    ready.add(key)
                zero_cols(t4, g, za, zb)

            def emit_left_pair(i):
                # covers d = (2i+1, 2i+2); tile A (i even) / B (i odd)
                d0 = 2 * i + 1
                j = i % 2
                for g, d in ((0, d0), (1, d0 + 1)):
                    za = 0 if ("p", j, g) not in ready else d - 4
                    prep_slot(lp3[j], ("p", j, g), g, za, d, eng="vector")
                nc.gpsimd.dma_start(dest(left_out, d0 - MIN_D, 2), lp[j][:])

            def emit_left_single39():
                d = 39  # tile B slot0: previous use d=35, zero [35,39)
                prep_slot(lp3[1], ("p", 1, 0), 0, d - 4, d)
                nc.gpsimd.dma_start(
                    left_out.ap()[d - MIN_D], lp[1][:, 0 : HL * W]
                )

            def emit_neg_pair(i):
                # covers d = (-2i-2, -2i-1) ascending; tile C/D; slot0
                # holds the more-negative d so dest d stays ascending.
                d0 = -2 * i - 2
                j = i % 2
                for g, d in ((0, d0), (1, d0 + 1)):
                    if ("n", j, g) not in ready:
                        prep_slot(ln3[j], ("n", j, g), g, W + d, W)
                    else:
                        zero_cols(ln3[j], g, W + d, W + d + 4)
                nc.gpsimd.dma_start(dest(left_out, d0 - MIN_D, 2), ln[j][:])

            def emit_right_batch(di0, g):
                st = stpool.tile([NPART, 4 * HL * W], f32, tag="st")
                st4 = st[:].rearrange("p (g h w) -> p g h w", g=4, h=HL)
                for k in range(g):
                    a = PAD_L - (di0 + k + MIN_D)
                    nc.vector.tensor_copy(st4[:, k, :, :], rp3[:, :, a : a + W])
                nc.gpsimd.dma_start(
                    dest(right_out, di0, g),
                    st[:, 0 : g * HL * W],
                )

            # ---- emission schedule ----
            # The gpsimd queue dispatches in order, so each dma_start
            # is placed after the work that unblocks it is plausibly
            # done: P0 needs only the left image + DVE copies; right
            # batches need the right image + DVE staging; neg pairs
            # need ACT copies. di=8 (d=0) is the DRAM->DRAM prologue.
            emit_left_pair(0)          # d 1,2
            emit_right_batch(0, 2)
            emit_left_pair(1)          # d 3,4
            emit_right_batch(2, 2)
            emit_neg_pair(0)           # d -2,-1
            emit_right_batch(4, 2)
            emit_neg_pair(1)           # d -4,-3
            emit_right_batch(6, 2)
            rights = [(9, 4), (13, 4), (17, 4), (21, 4), (25, 4), (29, 4),
                      (33, 4), (37, 4), (41, 4), (45, 3)]
            lefts = (
                [("P", i) for i in range(2, 19)]
                + [("S", None), ("N", 2), ("N", 3)]
            )
            li, ri = 0, 0
            while li < len(lefts) or ri < len(rights):
                if ri < len(rights):
                    emit_right_batch(*rights[ri])
                    ri += 1
                for _ in range(2):
                    if li < len(lefts):
                        kind, i = lefts[li]
                        if kind == "P":
                            emit_left_pair(i)
                        elif kind == "S":
                            emit_left_single39()
                        else:
                            emit_neg_pair(i)
                        li += 1

    nc.compile()
    return nc


def _get_nc():
    if "nc" not in _CACHE:
        _CACHE["nc"] = _build_nc()
    return _CACHE["nc"]


def kernel(left_feat, right_feat):
    from concourse.bass_utils import run_bass_kernel_spmd

    left = np.ascontiguousarray(np.asarray(left_feat), dtype=np.float32)
    right = np.ascontiguousarray(np.asarray(right_feat), dtype=np.float32)
    assert left.shape == (B, C, H, W) and right.shape == (B, C, H, W)

    nc = _get_nc()
    left_pad = np.zeros((B, C, H, WP), dtype=np.float32)
    left_pad[:, :, :, PAD_L : PAD_L + W] = left
    right_pad = np.zeros((B, C, H, WP), dtype=np.float32)
    right_pad[:, :, :, PAD_L : PAD_L + W] = right
    in_maps = []
    for m in range(N_CORES):
        sl = slice(m * CPC, (m + 1) * CPC)
        in_maps.append(
            {
                "left_in": np.ascontiguousarray(left_pad[:, sl]),
                "right_in": np.ascontiguousarray(right_pad[:, sl]),
                "left_raw": np.ascontiguousarray(left[:, sl]),
                "right_raw": np.ascontiguousarray(right[:, sl]),
            }
        )
    res = run_bass_kernel_spmd(nc, in_maps, core_ids=list(range(N_CORES))).results

    out = np.empty((B, 2 * C, D, H, W), dtype=np.float32)
    for m in range(N_CORES):
        sl = slice(m * CPC, (m + 1) * CPC)
        out[:, sl] = res[m]["left_out"].transpose(1, 2, 0, 3, 4)
        out[:, C + m * CPC : C + (m + 1) * CPC] = res[m]["right_out"].transpose(
            1, 2, 0, 3, 4
        )
    return out
